# revision 26
# baseline (speedup 1.0000x reference)
"""CenterNCALoss kernel for Trainium2 (8 NeuronCores, Bass).

Reference computation (f32):
    counts  = segment_sum(ones, targets)                  # [C]
    centers = segment_sum(inputs, targets) / counts       # [C, D]
    dist    = xx + yy - 2 * inputs @ centers.T            # [N, C]
    pos     = dist[i, targets[i]]
    base    = (max(dist) + min(dist)) / 2
    loss    = -mean(log(exp(-a*(pos-base)) / sum_c exp(-a*(dist-base))))
    dist_an = (sum(dist) - sum(pos)) / (N*(C-1))
    dist_ap = (ni/(ni-1)) * mean(pos)

Key algebraic identities (exact, any targets):
    sum(dist) = C*sum(xx) + N*sum(yy) - 2 * (sum_i x_i) . (sum_c c_c)
    sum(pos)  = sum(xx) - sum_c counts_c * yy_c
      [since sum_{i:t_i=c} x_i = counts_c * c_c  =>  sum_i x_i.c_{t_i} = sum_c counts_c*yy_c]

So dist_ap / dist_an need only per-class sums and sum of squares: O(N*D),
memory-bound.  The loss term with ALPHA=16 saturates f32: dist has spread
~150, so exp(-a*(dist-base)) overflows to inf on rows where dist < base-5.55,
giving log(inf/inf) = nan deterministically.  We prove nan-ness rigorously on
the host (see _loss_is_nan: dist >= 0 => base >= max(dist)/2; any sampled row
with pos < base - 5.55 has both pos_exp = inf and a_exp = inf since the pos
term is a summand of a_exp => that row's log is nan => the mean is nan).  If
the proof condition ever fails, we fall back to a faithful f32 recomputation.

Device kernel (per core, slab of ROWS=8192 rows):
    A 1024-row block has class(row) = row mod 1024.  SBUF tile [128, (j,d)]
    with partition p holding rows 8p..8p+7 of the block => class = 8p + j and
    per-partition DMA chunks are 4 KiB contiguous (4 KiB is the max the class
    layout allows; smaller chunks are DMA-descriptor-rate-bound at ~235 GB/s
    vs ~400 GB/s here).  Loads stream on the SP HWDGE ring, the last tile as
    two halves so its semaphores fire sooner.  Per-class partial sums = two
    independent half-chains of tensor_adds on DVE (so each half-store can
    issue as soon as its chain closes); sum of squares via activation(Square,
    accum_out) on ACT with the Square LUT prefetched by a dummy activate.
    Stores go out on two HWDGE rings in parallel (SP: csum half 0; ACT: csum
    half 1 + xx partials).  GpSimd/PE stay idle: fp32 matmul is 2-pass
    (LOW_HIGH) so PE-based reduction loses, and GpSimd streaming ops stall
    DVE via the shared SBUF port pair.  Outputs per core: csum [1024,128]
    partial class sums and xxp [128, QB+1] partial sums of squares.
    Measured: ~27-30 us per core end-to-end (NTFF), ~11 us of which is the
    4 MiB HBM read at roofline and ~10 us fixed NEFF preamble/epilogue.
"""

import sys

if "/opt/trn_rl_repo" not in sys.path:
    sys.path.insert(0, "/opt/trn_rl_repo")

from contextlib import ExitStack

import numpy as np

ALPHA = 16.0
N, D, C = 65536, 128, 1024
NCORES = 8
ROWS = N // NCORES  # 8192 rows per core
QB = ROWS // C      # 8 blocks of C rows per core
J = C // 128        # 8 classes per partition

_NC_CACHE = {}


def _build_nc():
    import concourse.bass as bass
    from concourse import mybir

    nc = bass.Bass(
        "TRN2", num_devices=NCORES, enable_partition_id=False, enable_asserts=False
    )
    f32 = mybir.dt.float32
    H = 512  # half width for the split store

    x = nc.declare_dram_parameter("x", [ROWS, D], f32, isOutput=False)
    csum = nc.declare_dram_parameter("csum", [C, D], f32, isOutput=True)
    xxp = nc.declare_dram_parameter("xxp", [128, QB + 1], f32, isOutput=True)

    # x[q*1024 + 8p + j, d] -> block q, partition p, free (j*128 + d)
    xr = x.rearrange("(q p j) d -> q p (j d)", q=QB, p=128, j=J)
    # csum[8p + j, d] <- partition p, free (j*128 + d)
    csum_r = csum.rearrange("(p j) d -> p (j d)", p=128, j=J)

    with ExitStack() as ctx:
        tiles = [
            ctx.enter_context(nc.sbuf_tensor(f"t{q}", [128, J * D], f32))
            for q in range(QB)
        ]
        S = ctx.enter_context(nc.sbuf_tensor("S", [128, J * D], f32))
        scr_a = ctx.enter_context(nc.sbuf_tensor("scr_a", [128, J * D], f32))
        xxs = ctx.enter_context(nc.sbuf_tensor("xxs", [128, QB + 1], f32))

        in_sems = [ctx.enter_context(nc.semaphore(f"in{q}")) for q in range(QB)]
        in7h_sem = ctx.enter_context(nc.semaphore("in7h"))
        ve_sems = [ctx.enter_context(nc.semaphore(f"ve{h}")) for h in range(2)]
        so_sem = ctx.enter_context(nc.semaphore("so"))
        ao_sem = ctx.enter_context(nc.semaphore("ao"))
        block = ctx.enter_context(nc.Block())

        LAST = QB - 1

        @block.sync
        def _(sync):
            # full tiles (4KiB chunks) on the SP HWDGE ring; the last tile
            # as two halves so its sems fire sooner
            for q in range(QB - 1):
                sync.dma_start(out=tiles[q][:], in_=xr[q]).then_inc(in_sems[q], 16)
            sync.dma_start(out=tiles[LAST][:, 0:H], in_=xr[LAST][:, 0:H]).then_inc(
                in7h_sem, 16
            )
            sync.dma_start(out=tiles[LAST][:, H:], in_=xr[LAST][:, H:]).then_inc(
                in_sems[LAST], 16
            )
            # store class-sums half 0 once its chain is done
            sync.wait_ge(ve_sems[0], 1)
            sync.dma_start(out=csum_r[:, 0:H], in_=S[:, 0:H]).then_inc(so_sem, 16)
            sync.wait_ge(so_sem, 16)

        @block.vector
        def _(vector):
            # two independent half-chains so each half-store can go early
            vector.wait_ge(in_sems[0], 16)
            vector.wait_ge(in_sems[1], 16)
            vector.tensor_add(S[:, 0:H], tiles[0][:, 0:H], tiles[1][:, 0:H])
            vector.tensor_add(S[:, H:], tiles[0][:, H:], tiles[1][:, H:])
            for q in range(2, QB - 1):
                vector.wait_ge(in_sems[q], 16)
                vector.tensor_add(S[:, 0:H], S[:, 0:H], tiles[q][:, 0:H])
                vector.tensor_add(S[:, H:], S[:, H:], tiles[q][:, H:])
            vector.wait_ge(in7h_sem, 16)
            vector.tensor_add(S[:, 0:H], S[:, 0:H], tiles[LAST][:, 0:H]).then_inc(
                ve_sems[0], 1
            )
            vector.wait_ge(in_sems[LAST], 16)
            vector.tensor_add(S[:, H:], S[:, H:], tiles[LAST][:, H:]).then_inc(
                ve_sems[1], 1
            )

        @block.scalar
        def _(scalar):
            # dummy activate to prefetch the Square LUT during the DMA wait
            scalar.activation(
                scr_a[:, 0:1], xxs[:, 0:1], mybir.ActivationFunctionType.Square
            )
            for q in range(QB - 1):
                scalar.wait_ge(in_sems[q], 16)
                scalar.activation(
                    scr_a[:, :],
                    tiles[q][:],
                    mybir.ActivationFunctionType.Square,
                    accum_out=xxs[:, q : q + 1],
                )
            scalar.wait_ge(in7h_sem, 16)
            scalar.activation(
                scr_a[:, 0:H],
                tiles[LAST][:, 0:H],
                mybir.ActivationFunctionType.Square,
                accum_out=xxs[:, LAST : LAST + 1],
            )
            scalar.wait_ge(in_sems[LAST], 16)
            scalar.activation(
                scr_a[:, H:],
                tiles[LAST][:, H:],
                mybir.ActivationFunctionType.Square,
                accum_out=xxs[:, QB : QB + 1],
            )
            # stores on the ACT HWDGE ring: class-sums half 1, then xx partials
            scalar.wait_ge(ve_sems[1], 1)
            scalar.dma_start(out=csum_r[:, H:], in_=S[:, H:]).then_inc(ao_sem, 16)
            scalar.dma_start(out=xxp[:, :], in_=xxs[:]).then_inc(ao_sem, 16)
            scalar.wait_ge(ao_sem, 32)

    return nc


def _get_nc():
    if "nc" not in _NC_CACHE:
        _NC_CACHE["nc"] = _build_nc()
    return _NC_CACHE["nc"]


def _run_device(x32, trace=False):
    """Run the SPMD bass kernel; returns (S_total[f64 C,D], sum_xx[f64], results)."""
    from concourse.bass_utils import run_bass_kernel_spmd

    nc = _get_nc()
    core_ids = list(range(NCORES))
    in_maps = [
        {"x": np.ascontiguousarray(x32[k * ROWS : (k + 1) * ROWS])} for k in core_ids
    ]
    res = run_bass_kernel_spmd(nc, in_maps, core_ids, trace=trace)
    S_total = np.zeros((C, D), dtype=np.float64)
    sum_xx = 0.0
    for k in core_ids:
        S_total += res.results[k]["csum"].astype(np.float64)
        sum_xx += float(res.results[k]["xxp"].astype(np.float64).sum())
    return S_total, sum_xx, res


def _loss_is_nan(x32, tgt, centers, yy, n_probe=2048):
    """Rigorous sufficient condition for the reference f32 loss being nan.

    dist >= -0.1 (squared distances; f32 rounding slack) => base >= max/2-0.05.
    For any row with pos < base - 5.8 (alpha*(base-pos) > 92.8 > log(f32max)):
    pos_exp = exp(+big) = inf, and a_exp >= its pos summand = inf, so the
    row's log(pos_exp/a_exp) = log(inf/inf) = nan and the mean is nan.
    max(dist) >= max over any probed subset, so this check is conservative.
    """
    idx = np.arange(0, N, max(1, N // n_probe))
    xs = x32[idx].astype(np.float64)
    dd = (xs * xs).sum(1)[:, None] + yy[None, :] - 2.0 * xs @ centers.T
    pos = dd[np.arange(len(idx)), tgt[idx]]
    base_lb = dd.max() / 2.0 - 0.05
    return bool(pos.min() < base_lb - 5.8)


def _faithful_f32(x32, tgt):
    """Full f32 numpy mirror of the reference (fallback path)."""
    counts = np.bincount(tgt, minlength=C).astype(np.float32)
    centers = np.zeros((C, D), dtype=np.float32)
    np.add.at(centers, tgt, x32)
    centers = centers / counts[:, None]
    xx = (x32 * x32).sum(1, dtype=np.float32)
    yy = (centers * centers).sum(1, dtype=np.float32)

    chunk = 4096
    gmax, gmin = -np.inf, np.inf
    for s in range(0, N, chunk):
        d_ = xx[s : s + chunk, None] + yy[None, :] - 2.0 * (
            x32[s : s + chunk] @ centers.T
        )
        gmax = max(gmax, float(d_.max()))
        gmin = min(gmin, float(d_.min()))
    base = np.float32((gmax + gmin) * 0.5)

    a = np.float32(ALPHA)
    tot = np.float64(0.0)
    sum_dist = 0.0
    sum_pos = 0.0
    with np.errstate(over="ignore", divide="ignore", invalid="ignore"):
        for s in range(0, N, chunk):
            d_ = xx[s : s + chunk, None] + yy[None, :] - 2.0 * (
                x32[s : s + chunk] @ centers.T
            )
            pos = np.take_along_axis(d_, tgt[s : s + chunk, None], axis=1)[:, 0]
            pe = np.exp(-a * (pos - base))
            ae = np.exp(-a * (d_ - base)).sum(1, dtype=np.float32)
            tot += np.log(pe / ae).astype(np.float64).sum()
            sum_dist += float(d_.sum(dtype=np.float64))
            sum_pos += float(pos.sum(dtype=np.float64))
    loss = np.float32(-(tot / N))
    ni = N // C
    dist_ap = np.float32((float(ni) / (ni - 1)) * (sum_pos / N))
    dist_an = np.float32((sum_dist - sum_pos) / (N * (C - 1)))
    return loss, 0, dist_ap, dist_an


def kernel(**inputs):
    x = np.asarray(inputs["inputs"])
    tgt = np.asarray(inputs["targets"]).astype(np.int64)
    x32 = np.ascontiguousarray(x, dtype=np.float32)

    arange_targets = bool(
        x.shape == (N, D) and np.array_equal(tgt, np.arange(N, dtype=np.int64) % C)
    )
    if not arange_targets:
        return _faithful_f32(x32, tgt.astype(np.int64))

    S_total, sum_xx, _ = _run_device(x32)

    counts = np.full(C, N // C, dtype=np.float64)  # validated arange targets
    centers = S_total / counts[:, None]
    yy = (centers * centers).sum(1)
    sum_yy = float(yy.sum())
    wyy = float((counts * yy).sum())
    sx = S_total.sum(0)
    cs = centers.sum(0)

    sum_dist = C * sum_xx + N * sum_yy - 2.0 * float(sx @ cs)
    sum_pos = sum_xx - wyy
    ni = N // C
    dist_ap = np.float32((float(ni) / (ni - 1)) * (sum_pos / N))
    dist_an = np.float32((sum_dist - sum_pos) / (N * (C - 1)))

    if _loss_is_nan(x32, tgt, centers, yy):
        loss = np.float32(np.nan)
    else:  # pragma: no cover - never taken for this data regime
        loss = _faithful_f32(x32, tgt)[0]

    return loss, 0, dist_ap, dist_an


# revision 27
# speedup vs baseline: 1.0316x; 1.0316x over previous
"""CenterNCALoss kernel for Trainium2 (8 NeuronCores, Bass).

Reference computation (f32):
    counts  = segment_sum(ones, targets)                  # [C]
    centers = segment_sum(inputs, targets) / counts       # [C, D]
    dist    = xx + yy - 2 * inputs @ centers.T            # [N, C]
    pos     = dist[i, targets[i]]
    base    = (max(dist) + min(dist)) / 2
    loss    = -mean(log(exp(-a*(pos-base)) / sum_c exp(-a*(dist-base))))
    dist_an = (sum(dist) - sum(pos)) / (N*(C-1))
    dist_ap = (ni/(ni-1)) * mean(pos)

Key algebraic identities (exact, any targets):
    sum(dist) = C*sum(xx) + N*sum(yy) - 2 * (sum_i x_i) . (sum_c c_c)
    sum(pos)  = sum(xx) - sum_c counts_c * yy_c
      [since sum_{i:t_i=c} x_i = counts_c * c_c  =>  sum_i x_i.c_{t_i} = sum_c counts_c*yy_c]

So dist_ap / dist_an need only per-class sums and sum of squares: O(N*D),
memory-bound.  The loss term with ALPHA=16 saturates f32: dist has spread
~150, so exp(-a*(dist-base)) overflows to inf on rows where dist < base-5.55,
giving log(inf/inf) = nan deterministically.  We prove nan-ness rigorously on
the host (see _loss_is_nan: dist >= 0 => base >= max(dist)/2; any sampled row
with pos < base - 5.55 has both pos_exp = inf and a_exp = inf since the pos
term is a summand of a_exp => that row's log is nan => the mean is nan).  If
the proof condition ever fails, we fall back to a faithful f32 recomputation.

Device kernel (per core, slab of ROWS=8192 rows):
    A 1024-row block has class(row) = row mod 1024.  SBUF tile [128, (j,d)]
    with partition p holding rows 8p..8p+7 of the block => class = 8p + j and
    per-partition DMA chunks are 4 KiB contiguous (4 KiB is the max the class
    layout allows; smaller chunks are DMA-descriptor-rate-bound at ~235 GB/s
    vs ~400 GB/s here).  Loads stream on the SP HWDGE ring, the last tile as
    two halves so its semaphores fire sooner.  Per-class partial sums = two
    independent half-chains of tensor_adds on DVE (so each half-store can
    issue as soon as its chain closes); sum of squares via activation(Square,
    accum_out) on ACT with the Square LUT prefetched by a dummy activate.
    Stores go out on two HWDGE rings in parallel (SP: csum half 0; ACT: csum
    half 1 + xx partials).  GpSimd/PE stay idle: fp32 matmul is 2-pass
    (LOW_HIGH) so PE-based reduction loses, and GpSimd streaming ops stall
    DVE via the shared SBUF port pair.  Outputs per core: csum [1024,128]
    partial class sums and xxp [128, QB+1] partial sums of squares.
    Measured: ~27-30 us per core end-to-end (NTFF), ~11 us of which is the
    4 MiB HBM read at roofline and ~10 us fixed NEFF preamble/epilogue.
"""

import sys

if "/opt/trn_rl_repo" not in sys.path:
    sys.path.insert(0, "/opt/trn_rl_repo")

from contextlib import ExitStack

import numpy as np

ALPHA = 16.0
N, D, C = 65536, 128, 1024
NCORES = 8
ROWS = N // NCORES  # 8192 rows per core
QB = ROWS // C      # 8 blocks of C rows per core
J = C // 128        # 8 classes per partition

_NC_CACHE = {}


def _build_nc():
    import concourse.bass as bass
    from concourse import mybir

    nc = bass.Bass(
        "TRN2", num_devices=NCORES, enable_partition_id=False, enable_asserts=False
    )
    f32 = mybir.dt.float32
    H = 512  # half width for the split store

    x = nc.declare_dram_parameter("x", [ROWS, D], f32, isOutput=False)
    csum = nc.declare_dram_parameter("csum", [C, D], f32, isOutput=True)
    xxp = nc.declare_dram_parameter("xxp", [128, QB + 1], f32, isOutput=True)

    # x[q*1024 + 8p + j, d] -> block q, partition p, free (j*128 + d)
    xr = x.rearrange("(q p j) d -> q p (j d)", q=QB, p=128, j=J)
    # csum[8p + j, d] <- partition p, free (j*128 + d)
    csum_r = csum.rearrange("(p j) d -> p (j d)", p=128, j=J)

    with ExitStack() as ctx:
        tiles = [
            ctx.enter_context(nc.sbuf_tensor(f"t{q}", [128, J * D], f32))
            for q in range(QB)
        ]
        S = ctx.enter_context(nc.sbuf_tensor("S", [128, J * D], f32))
        scr_a = ctx.enter_context(nc.sbuf_tensor("scr_a", [128, J * D], f32))
        xxs = ctx.enter_context(nc.sbuf_tensor("xxs", [128, QB + 1], f32))

        in_sems = [ctx.enter_context(nc.semaphore(f"in{q}")) for q in range(QB)]
        in7h_sem = ctx.enter_context(nc.semaphore("in7h"))
        ve_sems = [ctx.enter_context(nc.semaphore(f"ve{h}")) for h in range(2)]
        so_sem = ctx.enter_context(nc.semaphore("so"))
        ao_sem = ctx.enter_context(nc.semaphore("ao"))
        block = ctx.enter_context(nc.Block())

        LAST = QB - 1

        @block.sync
        def _(sync):
            # full tiles (4KiB chunks) on the SP HWDGE ring; the last tile
            # as two halves so its sems fire sooner
            for q in range(QB - 1):
                sync.dma_start(out=tiles[q][:], in_=xr[q]).then_inc(in_sems[q], 16)
            sync.dma_start(out=tiles[LAST][:, 0:H], in_=xr[LAST][:, 0:H]).then_inc(
                in7h_sem, 16
            )
            # store class-sums half 0 once its chain is done
            sync.wait_ge(ve_sems[0], 1)
            sync.dma_start(out=csum_r[:, 0:H], in_=S[:, 0:H]).then_inc(so_sem, 16)
            sync.wait_ge(so_sem, 16)

        @block.vector
        def _(vector):
            # two independent half-chains so each half-store can go early
            vector.wait_ge(in_sems[0], 16)
            vector.wait_ge(in_sems[1], 16)
            vector.tensor_add(S[:, 0:H], tiles[0][:, 0:H], tiles[1][:, 0:H])
            vector.tensor_add(S[:, H:], tiles[0][:, H:], tiles[1][:, H:])
            for q in range(2, QB - 1):
                vector.wait_ge(in_sems[q], 16)
                vector.tensor_add(S[:, 0:H], S[:, 0:H], tiles[q][:, 0:H])
                vector.tensor_add(S[:, H:], S[:, H:], tiles[q][:, H:])
            vector.wait_ge(in7h_sem, 16)
            vector.tensor_add(S[:, 0:H], S[:, 0:H], tiles[LAST][:, 0:H]).then_inc(
                ve_sems[0], 1
            )
            vector.wait_ge(in_sems[LAST], 16)
            vector.tensor_add(S[:, H:], S[:, H:], tiles[LAST][:, H:]).then_inc(
                ve_sems[1], 1
            )

        @block.scalar
        def _(scalar):
            # dummy activate to prefetch the Square LUT during the DMA wait
            scalar.activation(
                scr_a[:, 0:1], xxs[:, 0:1], mybir.ActivationFunctionType.Square
            )
            for q in range(QB - 1):
                scalar.wait_ge(in_sems[q], 16)
                scalar.activation(
                    scr_a[:, :],
                    tiles[q][:],
                    mybir.ActivationFunctionType.Square,
                    accum_out=xxs[:, q : q + 1],
                )
                if q == 3:
                    scalar.dma_start(
                        out=tiles[LAST][:, H:], in_=xr[LAST][:, H:]
                    ).then_inc(in_sems[LAST], 16)
            scalar.wait_ge(in7h_sem, 16)
            scalar.activation(
                scr_a[:, 0:H],
                tiles[LAST][:, 0:H],
                mybir.ActivationFunctionType.Square,
                accum_out=xxs[:, LAST : LAST + 1],
            )
            scalar.wait_ge(in_sems[LAST], 16)
            scalar.activation(
                scr_a[:, H:],
                tiles[LAST][:, H:],
                mybir.ActivationFunctionType.Square,
                accum_out=xxs[:, QB : QB + 1],
            )
            # stores on the ACT HWDGE ring: class-sums half 1, then xx partials
            scalar.wait_ge(ve_sems[1], 1)
            scalar.dma_start(out=csum_r[:, H:], in_=S[:, H:]).then_inc(ao_sem, 16)
            scalar.dma_start(out=xxp[:, :], in_=xxs[:]).then_inc(ao_sem, 16)
            scalar.wait_ge(ao_sem, 32)

    return nc


def _get_nc():
    if "nc" not in _NC_CACHE:
        _NC_CACHE["nc"] = _build_nc()
    return _NC_CACHE["nc"]


def _run_device(x32, trace=False):
    """Run the SPMD bass kernel; returns (S_total[f64 C,D], sum_xx[f64], results)."""
    from concourse.bass_utils import run_bass_kernel_spmd

    nc = _get_nc()
    core_ids = list(range(NCORES))
    in_maps = [
        {"x": np.ascontiguousarray(x32[k * ROWS : (k + 1) * ROWS])} for k in core_ids
    ]
    res = run_bass_kernel_spmd(nc, in_maps, core_ids, trace=trace)
    S_total = np.zeros((C, D), dtype=np.float64)
    sum_xx = 0.0
    for k in core_ids:
        S_total += res.results[k]["csum"].astype(np.float64)
        sum_xx += float(res.results[k]["xxp"].astype(np.float64).sum())
    return S_total, sum_xx, res


def _loss_is_nan(x32, tgt, centers, yy, n_probe=2048):
    """Rigorous sufficient condition for the reference f32 loss being nan.

    dist >= -0.1 (squared distances; f32 rounding slack) => base >= max/2-0.05.
    For any row with pos < base - 5.8 (alpha*(base-pos) > 92.8 > log(f32max)):
    pos_exp = exp(+big) = inf, and a_exp >= its pos summand = inf, so the
    row's log(pos_exp/a_exp) = log(inf/inf) = nan and the mean is nan.
    max(dist) >= max over any probed subset, so this check is conservative.
    """
    idx = np.arange(0, N, max(1, N // n_probe))
    xs = x32[idx].astype(np.float64)
    dd = (xs * xs).sum(1)[:, None] + yy[None, :] - 2.0 * xs @ centers.T
    pos = dd[np.arange(len(idx)), tgt[idx]]
    base_lb = dd.max() / 2.0 - 0.05
    return bool(pos.min() < base_lb - 5.8)


def _faithful_f32(x32, tgt):
    """Full f32 numpy mirror of the reference (fallback path)."""
    counts = np.bincount(tgt, minlength=C).astype(np.float32)
    centers = np.zeros((C, D), dtype=np.float32)
    np.add.at(centers, tgt, x32)
    centers = centers / counts[:, None]
    xx = (x32 * x32).sum(1, dtype=np.float32)
    yy = (centers * centers).sum(1, dtype=np.float32)

    chunk = 4096
    gmax, gmin = -np.inf, np.inf
    for s in range(0, N, chunk):
        d_ = xx[s : s + chunk, None] + yy[None, :] - 2.0 * (
            x32[s : s + chunk] @ centers.T
        )
        gmax = max(gmax, float(d_.max()))
        gmin = min(gmin, float(d_.min()))
    base = np.float32((gmax + gmin) * 0.5)

    a = np.float32(ALPHA)
    tot = np.float64(0.0)
    sum_dist = 0.0
    sum_pos = 0.0
    with np.errstate(over="ignore", divide="ignore", invalid="ignore"):
        for s in range(0, N, chunk):
            d_ = xx[s : s + chunk, None] + yy[None, :] - 2.0 * (
                x32[s : s + chunk] @ centers.T
            )
            pos = np.take_along_axis(d_, tgt[s : s + chunk, None], axis=1)[:, 0]
            pe = np.exp(-a * (pos - base))
            ae = np.exp(-a * (d_ - base)).sum(1, dtype=np.float32)
            tot += np.log(pe / ae).astype(np.float64).sum()
            sum_dist += float(d_.sum(dtype=np.float64))
            sum_pos += float(pos.sum(dtype=np.float64))
    loss = np.float32(-(tot / N))
    ni = N // C
    dist_ap = np.float32((float(ni) / (ni - 1)) * (sum_pos / N))
    dist_an = np.float32((sum_dist - sum_pos) / (N * (C - 1)))
    return loss, 0, dist_ap, dist_an


def kernel(**inputs):
    x = np.asarray(inputs["inputs"])
    tgt = np.asarray(inputs["targets"]).astype(np.int64)
    x32 = np.ascontiguousarray(x, dtype=np.float32)

    arange_targets = bool(
        x.shape == (N, D) and np.array_equal(tgt, np.arange(N, dtype=np.int64) % C)
    )
    if not arange_targets:
        return _faithful_f32(x32, tgt.astype(np.int64))

    S_total, sum_xx, _ = _run_device(x32)

    counts = np.full(C, N // C, dtype=np.float64)  # validated arange targets
    centers = S_total / counts[:, None]
    yy = (centers * centers).sum(1)
    sum_yy = float(yy.sum())
    wyy = float((counts * yy).sum())
    sx = S_total.sum(0)
    cs = centers.sum(0)

    sum_dist = C * sum_xx + N * sum_yy - 2.0 * float(sx @ cs)
    sum_pos = sum_xx - wyy
    ni = N // C
    dist_ap = np.float32((float(ni) / (ni - 1)) * (sum_pos / N))
    dist_an = np.float32((sum_dist - sum_pos) / (N * (C - 1)))

    if _loss_is_nan(x32, tgt, centers, yy):
        loss = np.float32(np.nan)
    else:  # pragma: no cover - never taken for this data regime
        loss = _faithful_f32(x32, tgt)[0]

    return loss, 0, dist_ap, dist_an


# revision 28
# speedup vs baseline: 1.1202x; 1.0859x over previous
"""CenterNCALoss kernel for Trainium2 (8 NeuronCores, Bass).

Reference computation (f32):
    counts  = segment_sum(ones, targets)                  # [C]
    centers = segment_sum(inputs, targets) / counts       # [C, D]
    dist    = xx + yy - 2 * inputs @ centers.T            # [N, C]
    pos     = dist[i, targets[i]]
    base    = (max(dist) + min(dist)) / 2
    loss    = -mean(log(exp(-a*(pos-base)) / sum_c exp(-a*(dist-base))))
    dist_an = (sum(dist) - sum(pos)) / (N*(C-1))
    dist_ap = (ni/(ni-1)) * mean(pos)

Key algebraic identities (exact, any targets):
    sum(dist) = C*sum(xx) + N*sum(yy) - 2 * (sum_i x_i) . (sum_c c_c)
    sum(pos)  = sum(xx) - sum_c counts_c * yy_c
      [since sum_{i:t_i=c} x_i = counts_c * c_c  =>  sum_i x_i.c_{t_i} = sum_c counts_c*yy_c]

So dist_ap / dist_an need only per-class sums and sum of squares: O(N*D),
memory-bound.  The loss term with ALPHA=16 saturates f32: dist has spread
~150, so exp(-a*(dist-base)) overflows to inf on rows where dist < base-5.55,
giving log(inf/inf) = nan deterministically.  We prove nan-ness rigorously on
the host (see _loss_is_nan: dist >= 0 => base >= max(dist)/2; any sampled row
with pos < base - 5.55 has both pos_exp = inf and a_exp = inf since the pos
term is a summand of a_exp => that row's log is nan => the mean is nan).  If
the proof condition ever fails, we fall back to a faithful f32 recomputation.

Device kernel (per core, slab of ROWS=8192 rows):
    A 1024-row block has class(row) = row mod 1024.  SBUF tile [128, (j,d)]
    with partition p holding rows 8p..8p+7 of the block => class = 8p + j and
    per-partition DMA chunks are 4 KiB contiguous (4 KiB is the max the class
    layout allows; smaller chunks are DMA-descriptor-rate-bound at ~235 GB/s
    vs ~400 GB/s here).  Loads stream on the SP HWDGE ring, the last tile as
    two halves so its semaphores fire sooner.  Per-class partial sums = two
    independent half-chains of tensor_adds on DVE (so each half-store can
    issue as soon as its chain closes); sum of squares via activation(Square,
    accum_out) on ACT with the Square LUT prefetched by a dummy activate.
    Stores go out on two HWDGE rings in parallel (SP: csum half 0; ACT: csum
    half 1 + xx partials).  GpSimd/PE stay idle: fp32 matmul is 2-pass
    (LOW_HIGH) so PE-based reduction loses, and GpSimd streaming ops stall
    DVE via the shared SBUF port pair.  Outputs per core: csum [1024,128]
    partial class sums and xxp [128, QB+1] partial sums of squares.
    Measured: ~27-30 us per core end-to-end (NTFF), ~11 us of which is the
    4 MiB HBM read at roofline and ~10 us fixed NEFF preamble/epilogue.
"""

import sys

if "/opt/trn_rl_repo" not in sys.path:
    sys.path.insert(0, "/opt/trn_rl_repo")

from contextlib import ExitStack

import numpy as np

ALPHA = 16.0
N, D, C = 65536, 128, 1024
NCORES = 8
ROWS = N // NCORES  # 8192 rows per core
QB = ROWS // C      # 8 blocks of C rows per core
J = C // 128        # 8 classes per partition

_NC_CACHE = {}


def _build_nc():
    import concourse.bass as bass
    from concourse import mybir

    nc = bass.Bass(
        "TRN2", num_devices=NCORES, enable_partition_id=False, enable_asserts=False
    )
    f32 = mybir.dt.float32
    H = 512  # half width for the split store

    x = nc.declare_dram_parameter("x", [ROWS, D], f32, isOutput=False)
    csum = nc.declare_dram_parameter("csum", [C, D], f32, isOutput=True)
    xxp = nc.declare_dram_parameter("xxp", [128, QB + 1], f32, isOutput=True)

    # x[q*1024 + 8p + j, d] -> block q, partition p, free (j*128 + d)
    xr = x.rearrange("(q p j) d -> q p (j d)", q=QB, p=128, j=J)
    # csum[8p + j, d] <- partition p, free (j*128 + d)
    csum_r = csum.rearrange("(p j) d -> p (j d)", p=128, j=J)

    with ExitStack() as ctx:
        tiles = [
            ctx.enter_context(nc.sbuf_tensor(f"t{q}", [128, J * D], f32))
            for q in range(QB)
        ]
        S = ctx.enter_context(nc.sbuf_tensor("S", [128, J * D], f32))
        scr_a = ctx.enter_context(nc.sbuf_tensor("scr_a", [128, J * D], f32))
        xxs = ctx.enter_context(nc.sbuf_tensor("xxs", [128, QB + 1], f32))

        in_sems = [ctx.enter_context(nc.semaphore(f"in{q}")) for q in range(QB)]
        in7h_sem = ctx.enter_context(nc.semaphore("in7h"))
        ve_sems = [ctx.enter_context(nc.semaphore(f"ve{h}")) for h in range(2)]
        so_sem = ctx.enter_context(nc.semaphore("so"))
        ao_sem = ctx.enter_context(nc.semaphore("ao"))
        block = ctx.enter_context(nc.Block())

        LAST = QB - 1

        @block.sync
        def _(sync):
            # full tiles (4KiB chunks) on the SP HWDGE ring; the last tile
            # as two halves so its sems fire sooner
            for q in range(QB - 1):
                sync.dma_start(out=tiles[q][:], in_=xr[q]).then_inc(in_sems[q], 16)
            sync.dma_start(out=tiles[LAST][:, 0:H], in_=xr[LAST][:, 0:H]).then_inc(
                in7h_sem, 16
            )
            sync.dma_start(out=tiles[LAST][:, H:], in_=xr[LAST][:, H:]).then_inc(
                in_sems[LAST], 16
            )
            # store class-sums half 0 once its chain is done
            sync.wait_ge(ve_sems[0], 1)
            sync.dma_start(out=csum_r[:, 0:H], in_=S[:, 0:H]).then_inc(so_sem, 16)
            sync.wait_ge(so_sem, 16)

        @block.vector
        def _(vector):
            # two independent half-chains so each half-store can go early
            vector.wait_ge(in_sems[0], 16)
            vector.wait_ge(in_sems[1], 16)
            vector.tensor_add(S[:, 0:H], tiles[0][:, 0:H], tiles[1][:, 0:H])
            vector.tensor_add(S[:, H:], tiles[0][:, H:], tiles[1][:, H:])
            for q in range(2, QB - 1):
                vector.wait_ge(in_sems[q], 16)
                vector.tensor_add(S[:, 0:H], S[:, 0:H], tiles[q][:, 0:H])
                vector.tensor_add(S[:, H:], S[:, H:], tiles[q][:, H:])
            vector.wait_ge(in7h_sem, 16)
            vector.tensor_add(S[:, 0:H], S[:, 0:H], tiles[LAST][:, 0:H]).then_inc(
                ve_sems[0], 1
            )
            vector.wait_ge(in_sems[LAST], 16)
            vector.tensor_add(S[:, H:], S[:, H:], tiles[LAST][:, H:]).then_inc(
                ve_sems[1], 1
            )

        @block.scalar
        def _(scalar):
            # dummy activate to prefetch the Square LUT during the DMA wait
            scalar.activation(
                scr_a[:, 0:1], xxs[:, 0:1], mybir.ActivationFunctionType.Square
            )
            for q in range(QB - 1):
                scalar.wait_ge(in_sems[q], 16)
                scalar.activation(
                    scr_a[:, :],
                    tiles[q][:],
                    mybir.ActivationFunctionType.Square,
                    accum_out=xxs[:, q : q + 1],
                )
            scalar.wait_ge(in7h_sem, 16)
            scalar.activation(
                scr_a[:, 0:H],
                tiles[LAST][:, 0:H],
                mybir.ActivationFunctionType.Square,
                accum_out=xxs[:, LAST : LAST + 1],
            )
            scalar.wait_ge(in_sems[LAST], 16)
            scalar.activation(
                scr_a[:, H:],
                tiles[LAST][:, H:],
                mybir.ActivationFunctionType.Square,
                accum_out=xxs[:, QB : QB + 1],
            )
            # stores on the ACT HWDGE ring: class-sums half 1, then xx partials
            scalar.wait_ge(ve_sems[1], 1)
            scalar.dma_start(out=csum_r[:, H:], in_=S[:, H:]).then_inc(ao_sem, 16)
            scalar.dma_start(out=xxp[:, :], in_=xxs[:]).then_inc(ao_sem, 16)
            scalar.wait_ge(ao_sem, 32)

    return nc


def _get_nc():
    if "nc" not in _NC_CACHE:
        _NC_CACHE["nc"] = _build_nc()
    return _NC_CACHE["nc"]


def _run_device(x32, trace=False):
    """Run the SPMD bass kernel; returns (S_total[f64 C,D], sum_xx[f64], results)."""
    from concourse.bass_utils import run_bass_kernel_spmd

    nc = _get_nc()
    core_ids = list(range(NCORES))
    in_maps = [
        {"x": np.ascontiguousarray(x32[k * ROWS : (k + 1) * ROWS])} for k in core_ids
    ]
    res = run_bass_kernel_spmd(nc, in_maps, core_ids, trace=trace)
    S_total = np.zeros((C, D), dtype=np.float64)
    sum_xx = 0.0
    for k in core_ids:
        S_total += res.results[k]["csum"].astype(np.float64)
        sum_xx += float(res.results[k]["xxp"].astype(np.float64).sum())
    return S_total, sum_xx, res


def _loss_is_nan(x32, tgt, centers, yy, n_probe=2048):
    """Rigorous sufficient condition for the reference f32 loss being nan.

    dist >= -0.1 (squared distances; f32 rounding slack) => base >= max/2-0.05.
    For any row with pos < base - 5.8 (alpha*(base-pos) > 92.8 > log(f32max)):
    pos_exp = exp(+big) = inf, and a_exp >= its pos summand = inf, so the
    row's log(pos_exp/a_exp) = log(inf/inf) = nan and the mean is nan.
    max(dist) >= max over any probed subset, so this check is conservative.
    """
    idx = np.arange(0, N, max(1, N // n_probe))
    xs = x32[idx].astype(np.float64)
    dd = (xs * xs).sum(1)[:, None] + yy[None, :] - 2.0 * xs @ centers.T
    pos = dd[np.arange(len(idx)), tgt[idx]]
    base_lb = dd.max() / 2.0 - 0.05
    return bool(pos.min() < base_lb - 5.8)


def _faithful_f32(x32, tgt):
    """Full f32 numpy mirror of the reference (fallback path)."""
    counts = np.bincount(tgt, minlength=C).astype(np.float32)
    centers = np.zeros((C, D), dtype=np.float32)
    np.add.at(centers, tgt, x32)
    centers = centers / counts[:, None]
    xx = (x32 * x32).sum(1, dtype=np.float32)
    yy = (centers * centers).sum(1, dtype=np.float32)

    chunk = 4096
    gmax, gmin = -np.inf, np.inf
    for s in range(0, N, chunk):
        d_ = xx[s : s + chunk, None] + yy[None, :] - 2.0 * (
            x32[s : s + chunk] @ centers.T
        )
        gmax = max(gmax, float(d_.max()))
        gmin = min(gmin, float(d_.min()))
    base = np.float32((gmax + gmin) * 0.5)

    a = np.float32(ALPHA)
    tot = np.float64(0.0)
    sum_dist = 0.0
    sum_pos = 0.0
    with np.errstate(over="ignore", divide="ignore", invalid="ignore"):
        for s in range(0, N, chunk):
            d_ = xx[s : s + chunk, None] + yy[None, :] - 2.0 * (
                x32[s : s + chunk] @ centers.T
            )
            pos = np.take_along_axis(d_, tgt[s : s + chunk, None], axis=1)[:, 0]
            pe = np.exp(-a * (pos - base))
            ae = np.exp(-a * (d_ - base)).sum(1, dtype=np.float32)
            tot += np.log(pe / ae).astype(np.float64).sum()
            sum_dist += float(d_.sum(dtype=np.float64))
            sum_pos += float(pos.sum(dtype=np.float64))
    loss = np.float32(-(tot / N))
    ni = N // C
    dist_ap = np.float32((float(ni) / (ni - 1)) * (sum_pos / N))
    dist_an = np.float32((sum_dist - sum_pos) / (N * (C - 1)))
    return loss, 0, dist_ap, dist_an


def kernel(**inputs):
    x = np.asarray(inputs["inputs"])
    tgt = np.asarray(inputs["targets"]).astype(np.int64)
    x32 = np.ascontiguousarray(x, dtype=np.float32)

    arange_targets = bool(
        x.shape == (N, D) and np.array_equal(tgt, np.arange(N, dtype=np.int64) % C)
    )
    if not arange_targets:
        return _faithful_f32(x32, tgt.astype(np.int64))

    S_total, sum_xx, _ = _run_device(x32)

    counts = np.full(C, N // C, dtype=np.float64)  # validated arange targets
    centers = S_total / counts[:, None]
    yy = (centers * centers).sum(1)
    sum_yy = float(yy.sum())
    wyy = float((counts * yy).sum())
    sx = S_total.sum(0)
    cs = centers.sum(0)

    sum_dist = C * sum_xx + N * sum_yy - 2.0 * float(sx @ cs)
    sum_pos = sum_xx - wyy
    ni = N // C
    dist_ap = np.float32((float(ni) / (ni - 1)) * (sum_pos / N))
    dist_an = np.float32((sum_dist - sum_pos) / (N * (C - 1)))

    if _loss_is_nan(x32, tgt, centers, yy):
        loss = np.float32(np.nan)
    else:  # pragma: no cover - never taken for this data regime
        loss = _faithful_f32(x32, tgt)[0]

    return loss, 0, dist_ap, dist_an
